# revision 1
# baseline (speedup 1.0000x reference)
"""Tucker-style 3-mode contraction kernel for Trainium2 (8 NeuronCores).

Problem: x [1024*32*32*32] fp32, w0/w1/w2 [32,32] fp32.
  out[B,A,Bb,C] = sum_{a,b,c} x[B,a,b,c] w0[a,A] w1[b,Bb] w2[c,C]

Strategy (per core, data-parallel over batch, 128 batch elems/core):
  - Sub-tile: 4 batch elems ("groups" g) x full 32x32x32 tensor each
    -> SBUF tile [128 partitions = (g, mode), 1024 free].
  - Stationary weights are diag4 = kron(I4, w) [128,128]: one matmul
    contracts the partition-mode of all 4 groups at once (2x N=512).
  - Between stages, DVE StreamTranspose on dense 2D tiles (32x32 blocks)
    moves the inner free mode onto partitions; strided matmul rhs views
    pre-swap the free order so the right mode is inner:
      MM1 (w0): X [(g,a),(b,c)]        -> psum1 [(g,A),(b,c)]
      T1:                               -> sbuf z1t [(g,c),(b,A)]
      MM2 (w2, rhs viewed (A,b)):       -> psum2 [(g,C),(A,b)]
      T2:                               -> sbuf z2t [(g,b),(A,C)]
      MM3 (w1, rhs viewed (C,A)):       -> psum3 [(g,B),(C,A)]
      T3:                               -> sbuf z3t [(g,A),(C,B)]
      ACT copy (in viewed (B,C)):       -> Y [(g,A),(B,C)]   (dense DMA out)
  - Super-tiles of 4 sub-tiles (16 batch elems) give 2 MiB contiguous DMAs.
"""

import os

import numpy as np

N_CORES = 8
BATCH = 1024
F = 32  # factor dim
ELEM = F * F * F  # 32768 elems per batch element
B_PER_CORE = BATCH // N_CORES  # 128
G = 4  # batch groups per sub-tile (4*32 = 128 partitions)
S = 4  # sub-tiles per super-tile
T = B_PER_CORE // (G * S)  # 8 super-tiles per core
FF = F * F  # 1024

# "float32" (exact, PE 4 cyc/row) or "float32r" (TF32-like, 1 cyc/row)
MM_DTYPE = os.environ.get("KERNEL_MM_DTYPE", "float32")

_CACHE = {}


def build_program(mm_dtype=MM_DTYPE, repeat=1):
    key = (mm_dtype, repeat)
    if key in _CACHE:
        return _CACHE[key]

    import concourse.bacc as bacc
    import concourse.mybir as mybir
    import concourse.tile as tile

    f32 = mybir.dt.float32
    mmdt = getattr(mybir.dt, mm_dtype)

    nc = bacc.Bacc("TRN2", target_bir_lowering=False, debug=False,
                   num_devices=N_CORES)

    xs = nc.dram_tensor("xs", [T, S, G, F, FF], mmdt, kind="ExternalInput")
    w0d = nc.dram_tensor("w0d", [128, 128], mmdt, kind="ExternalInput")
    w1d = nc.dram_tensor("w1d", [128, 128], mmdt, kind="ExternalInput")
    w2d = nc.dram_tensor("w2d", [128, 128], mmdt, kind="ExternalInput")
    ys = nc.dram_tensor("ys", [T, S, G, F, FF], f32, kind="ExternalOutput")

    def mm(out_ap, lhsT_ap, rhs_ap):
        nc.tensor.matmul(out_ap, lhsT_ap, rhs_ap, start=True, stop=True)

    with tile.TileContext(nc) as tc:
        with (
            tc.tile_pool(name="consts", bufs=1) as cpool,
            tc.tile_pool(name="xp", bufs=2) as xp,
            tc.tile_pool(name="yp", bufs=2) as yp,
            tc.tile_pool(name="zt", bufs=3) as ztp,
            tc.tile_pool(name="ps1", bufs=2, space="PSUM") as ps1,
            tc.tile_pool(name="ps2", bufs=1, space="PSUM") as ps2,
            tc.tile_pool(name="ps3", bufs=1, space="PSUM") as ps3,
        ):
            w0t = cpool.tile([128, 128], mmdt)
            w1t = cpool.tile([128, 128], mmdt)
            w2t = cpool.tile([128, 128], mmdt)
            nc.sync.dma_start(out=w0t[:], in_=w0d[:])
            nc.sync.dma_start(out=w1t[:], in_=w1d[:])
            nc.sync.dma_start(out=w2t[:], in_=w2d[:])

            for t in range(T * repeat):
                t = t % T
                X = xp.tile([128, S, FF], mmdt)  # [(g,a), s, (b,c)]
                nc.sync.dma_start(
                    out=X[:], in_=xs[t].rearrange("s g a m -> (g a) s m"))
                Y = yp.tile([128, S, F, F], f32)  # [(g,A), s, B, C]
                for s in range(S):
                    # stage 1: contract a -> psum1 [(g,A), (b,c)]
                    z1 = ps1.tile([128, FF], mmdt, tag="z1")
                    mm(z1[:, 0:512], w0t[:], X[:, s, 0:512])
                    mm(z1[:, 512:1024], w0t[:], X[:, s, 512:1024])
                    # T1: -> [(g,c), (b,A)]
                    z1t = ztp.tile([128, FF], mmdt, tag="z1t")
                    nc.vector.transpose(out=z1t[:], in_=z1[:])
                    # stage 2: contract c; rhs viewed (A,b) -> psum2 [(g,C),(A,b)]
                    z1v = z1t[:].rearrange("p (b a) -> p a b", b=F, a=F)
                    z2 = ps2.tile([128, FF], mmdt, tag="z2")
                    mm(z2[:, 0:512], w2t[:], z1v[:, 0:F // 2, :])
                    mm(z2[:, 512:1024], w2t[:], z1v[:, F // 2:F, :])
                    # T2: -> [(g,b), (A,C)]
                    z2t = ztp.tile([128, FF], mmdt, tag="z2t")
                    nc.vector.transpose(out=z2t[:], in_=z2[:])
                    # stage 3: contract b; rhs viewed (C,A) -> psum3 [(g,B),(C,A)]
                    z2v = z2t[:].rearrange("p (a c) -> p c a", a=F, c=F)
                    z3 = ps3.tile([128, FF], f32, tag="z3")
                    mm(z3[:, 0:512], w1t[:], z2v[:, 0:F // 2, :])
                    mm(z3[:, 512:1024], w1t[:], z2v[:, F // 2:F, :])
                    # T3: -> z3t [(g,A), (C,B)]
                    z3t = ztp.tile([128, FF], f32, tag="z3t")
                    nc.vector.transpose(out=z3t[:], in_=z3[:])
                    # final free reorder (C,B) -> (B,C) on ScalarE
                    nc.scalar.copy(
                        out=Y[:, s],
                        in_=z3t[:].rearrange("p (c b) -> p b c", c=F, b=F))
                nc.scalar.dma_start(
                    out=ys[t].rearrange("s g a (b c) -> (g a) s b c", b=F, c=F),
                    in_=Y[:])

    nc.compile()
    _CACHE[key] = nc
    return nc


def _diag4(w):
    return np.kron(np.eye(G, dtype=np.float32), np.asarray(w, np.float32))


def make_in_maps(x, w0, w1, w2):
    x = np.ascontiguousarray(np.asarray(x, np.float32).reshape(-1))
    assert x.size == BATCH * ELEM
    shards = x.reshape(N_CORES, T, S, G, F, FF)
    w0d, w1d, w2d = _diag4(w0), _diag4(w1), _diag4(w2)
    return [
        {"xs": shards[i], "w0d": w0d, "w1d": w1d, "w2d": w2d}
        for i in range(N_CORES)
    ]


def kernel(x, w0, w1, w2, trace=False):
    from concourse.bass_utils import run_bass_kernel_spmd

    nc = build_program()
    in_maps = make_in_maps(x, w0, w1, w2)
    res = run_bass_kernel_spmd(nc, in_maps, core_ids=list(range(N_CORES)),
                               trace=trace)
    out = np.concatenate([res.results[i]["ys"].reshape(-1)
                          for i in range(N_CORES)])
    if trace:
        return out, res
    return out



# revision 8
# speedup vs baseline: 1.8864x; 1.8864x over previous
"""Tucker-style 3-mode contraction kernel for Trainium2 (8 NeuronCores).

Problem: x [1024*32*32*32] fp32, w0/w1/w2 [32,32] fp32.
  out[B,A,Bb,C] = sum_{a,b,c} x[B,a,b,c] w0[a,A] w1[b,Bb] w2[c,C]

Data-parallel over batch: 128 batch elems/core; sub-tile = 4 batch elems
("groups" g) x full 32x32x32 tensor -> [128 p = (g, mode), 1024 f].
Stationary weights are kron(I4, w) [128,128]; one matmul (2x N=512)
contracts the partition-inner mode of all 4 groups at once.

v2 chain (contract order c, b, a; intermediate z's in bf16 so the DVE
StreamTransposes run at 2x; psum drains double as free-dim reorders and
run on ACT / GPSIMD to keep DVE under the DMA roofline):

  X    [(g,a),(b,c)]  f32r   <- dense DMA in (super-tile 2 MiB)
  T0   DVE ST          -> xt  [(g,c),(b,a)]  f32r
  MM1  kron(w2) f32r   -> z1  [(g,C),(b,a)]  psum f32
  D1   ACT copy (b,a)->(a,b), cvt bf16 -> z1b [(g,C),(a,b)]
  T1   DVE ST (bf16 2x) -> z1t [(g,b),(a,C)]
  MM2  kron(w1) bf16   -> z2  [(g,B),(a,C)]  psum f32
  D2   Pool copy (a,C)->(C,a), cvt bf16 -> z2b [(g,B),(C,a)]
  T2   DVE ST (bf16 2x) -> z2t [(g,a),(C,B)]
  MM3  kron(w0) bf16   -> z3  [(g,A),(C,B)]  psum f32
  OUT  ACT copy (C,B)->(B,C) -> Y [(g,A),(B,C)] f32 -> dense DMA out
"""

import os

import numpy as np

N_CORES = 8
BATCH = 1024
F = 32  # factor dim
ELEM = F * F * F  # 32768 elems per batch element
B_PER_CORE = BATCH // N_CORES  # 128
G = 4  # batch groups per sub-tile (4*32 = 128 partitions)
S = 4  # sub-tiles per super-tile
T = B_PER_CORE // (G * S)  # 8 super-tiles per core
FF = F * F  # 1024

# intermediate/matmul dtype: "bfloat16" (1 cyc/row mm, 2x DVE) or "float32"
Z_DTYPE = os.environ.get("KERNEL_Z_DTYPE", "bfloat16")
X_DTYPE = Z_DTYPE  # kept for test.py printout compat

_CACHE = {}


def build_program(z_dtype=Z_DTYPE, repeat=1):
    key = (z_dtype, repeat)
    if key in _CACHE:
        return _CACHE[key]

    import concourse.bacc as bacc
    import concourse.mybir as mybir
    import concourse.tile as tile

    f32 = mybir.dt.float32
    zdt = getattr(mybir.dt, z_dtype)

    nc = bacc.Bacc("TRN2", target_bir_lowering=False, debug=False,
                   num_devices=N_CORES)

    xs = nc.dram_tensor("xs", [T, S, G, F, FF], f32, kind="ExternalInput")
    wk2 = nc.dram_tensor("wk2", [128, 128], zdt, kind="ExternalInput")
    wk1 = nc.dram_tensor("wk1", [128, 128], zdt, kind="ExternalInput")
    wk0 = nc.dram_tensor("wk0", [128, 128], zdt, kind="ExternalInput")
    ys = nc.dram_tensor("ys", [T, S, G, F, FF], f32, kind="ExternalOutput")

    def mm(out_ap, lhsT_ap, rhs_ap):
        nc.tensor.matmul(out_ap, lhsT_ap, rhs_ap, start=True, stop=True)

    with tile.TileContext(nc) as tc:
        with (
            tc.tile_pool(name="consts", bufs=1) as cpool,
            tc.tile_pool(name="xp", bufs=2) as xp,
            tc.tile_pool(name="xbp", bufs=2) as xbp,
            tc.tile_pool(name="xtp", bufs=2) as xtp,
            tc.tile_pool(name="z1bp", bufs=2) as z1bp,
            tc.tile_pool(name="z1tp", bufs=2) as z1tp,
            tc.tile_pool(name="z2bp", bufs=2) as z2bp,
            tc.tile_pool(name="z2tp", bufs=2) as z2tp,
            tc.tile_pool(name="yp", bufs=2) as yp,
            tc.tile_pool(name="ps1", bufs=2, space="PSUM") as ps1,
            tc.tile_pool(name="ps2", bufs=1, space="PSUM") as ps2,
            tc.tile_pool(name="ps3", bufs=1, space="PSUM") as ps3,
        ):
            wk2t = cpool.tile([128, 128], zdt)
            wk1t = cpool.tile([128, 128], zdt)
            wk0t = cpool.tile([128, 128], zdt)
            nc.sync.dma_start(out=wk2t[:], in_=wk2[:])
            nc.sync.dma_start(out=wk1t[:], in_=wk1[:])
            nc.sync.dma_start(out=wk0t[:], in_=wk0[:])

            for t in range(T * repeat):
                t = t % T
                X = xp.tile([128, S, FF], f32)  # [(g,a), s, (b,c)]
                nc.sync.dma_start(
                    out=X[:], in_=xs[t].rearrange("s g a m -> (g a) s m"))
                Y = yp.tile([128, S, F, F], f32)  # [(g,A), s, B, C]
                for s in range(S):
                    # C0: convert x to bf16 (GPSIMD, sbuf->sbuf)
                    xb = xbp.tile([128, FF], zdt, tag="xb")
                    nc.gpsimd.tensor_copy(out=xb[:], in_=X[:, s])
                    # T0: [(g,a),(b,c)] -> [(g,c),(b,a)]
                    xt = xtp.tile([128, FF], zdt, tag="xt")
                    nc.vector.transpose(out=xt[:], in_=xb[:])
                    # MM1: contract c -> z1 [(g,C),(b,a)]
                    z1 = ps1.tile([128, FF], f32, tag="z1")
                    mm(z1[:, 0:512], wk2t[:], xt[:, 0:512])
                    mm(z1[:, 512:1024], wk2t[:], xt[:, 512:1024])
                    # D1: reorder (b,a)->(a,b), cvt -> z1b [(g,C),(a,b)]
                    z1b = z1bp.tile([128, F, F], zdt, tag="z1b")
                    nc.scalar.copy(
                        out=z1b[:],
                        in_=z1[:].rearrange("p (b a) -> p a b", b=F, a=F))
                    # T1: -> z1t [(g,b),(a,C)]
                    z1t = z1tp.tile([128, FF], zdt, tag="z1t")
                    nc.vector.transpose(
                        out=z1t[:], in_=z1b[:].rearrange("p a b -> p (a b)"))
                    # MM2: contract b -> z2 [(g,B),(a,C)]
                    z2 = ps2.tile([128, FF], f32, tag="z2")
                    mm(z2[:, 0:512], wk1t[:], z1t[:, 0:512])
                    mm(z2[:, 512:1024], wk1t[:], z1t[:, 512:1024])
                    # D2: reorder (a,C)->(C,a), cvt -> z2b [(g,B),(C,a)]
                    z2b = z2bp.tile([128, F, F], zdt, tag="z2b")
                    nc.scalar.copy(
                        out=z2b[:],
                        in_=z2[:].rearrange("p (a c) -> p c a", a=F, c=F))
                    # T2: -> z2t [(g,a),(C,B)]
                    z2t = z2tp.tile([128, FF], zdt, tag="z2t")
                    nc.vector.transpose(
                        out=z2t[:], in_=z2b[:].rearrange("p c a -> p (c a)"))
                    # MM3: contract a -> z3 [(g,A),(C,B)]
                    z3 = ps3.tile([128, FF], f32, tag="z3")
                    mm(z3[:, 0:512], wk0t[:], z2t[:, 0:512])
                    mm(z3[:, 512:1024], wk0t[:], z2t[:, 512:1024])
                    # OUT: reorder (C,B)->(B,C) -> Y f32
                    nc.scalar.copy(
                        out=Y[:, s],
                        in_=z3[:].rearrange("p (c b) -> p b c", c=F, b=F))
                nc.scalar.dma_start(
                    out=ys[t].rearrange("s g a (b c) -> (g a) s b c", b=F, c=F),
                    in_=Y[:])

    nc.compile()
    _CACHE[key] = nc
    return nc


def _kron4(w, np_dtype):
    return np.kron(np.eye(G, dtype=np.float32),
                   np.asarray(w, np.float32)).astype(np_dtype)


def make_in_maps(x, w0, w1, w2, z_dtype=Z_DTYPE):
    import ml_dtypes
    zdt_np = np.dtype(ml_dtypes.bfloat16) if z_dtype == "bfloat16" \
        else np.dtype(np.float32)
    x = np.ascontiguousarray(np.asarray(x, np.float32).reshape(-1))
    assert x.size == BATCH * ELEM
    shards = x.reshape(N_CORES, T, S, G, F, FF)
    wk2 = _kron4(w2, zdt_np)
    wk1 = _kron4(w1, zdt_np)
    wk0 = _kron4(w0, zdt_np)
    return [
        {"xs": shards[i], "wk2": wk2, "wk1": wk1, "wk0": wk0}
        for i in range(N_CORES)
    ]


def kernel(x, w0, w1, w2, trace=False):
    from concourse.bass_utils import run_bass_kernel_spmd

    nc = build_program()
    in_maps = make_in_maps(x, w0, w1, w2)
    res = run_bass_kernel_spmd(nc, in_maps, core_ids=list(range(N_CORES)),
                               trace=trace)
    out = np.concatenate([res.results[i]["ys"].reshape(-1)
                          for i in range(N_CORES)])
    if trace:
        return out, res
    return out
